# revision 3
# baseline (speedup 1.0000x reference)
"""Trainium2 Bass kernel v2 for nn_Attention_55233279426826 (block-causal attention).

Reference computation (per batch b):
    xn = LayerNorm(x[b]) * gamma + beta
    q,k,v = split(xn @ w_qkv), 12 heads x 64
    attn  = softmax(block-causal-masked(q k^T / 8))
    out[b] = (attn v) @ w_out + b_out

Sharding (8 cores): batch (2) x head-group (4, 3 heads each).  Host folds
gamma into w_qkv, precomputes beta @ w_qkv, permutes/casts weights to bf16,
sums the 4 head-group partials per batch and adds b_out.

v2 design notes (vs the 254us baseline):
  - All transposes are REGULAR bf16 matmuls against an identity rhs (not
    is_transpose), so they pipeline like matmuls and count as PE activity
    for the HAM clock gate (the baseline ran the whole attention phase at
    K=4/8 = 1.2 GHz).
  - qkv layout places q2/k2 at partition offset 64 so the three per-J score
    matmuls split 1/2 across PE row halves (tile_position auto-inferred).
  - Scores for the 3 heads land in one 3-bank psum tile [128,1536] and are
    exp'ed by ONE ACTIVATE per even J (odd J: two, reusing the h2 bank).
  - Softmax denominators: DVE reciprocal of the psum den row, partition-
    broadcast via a stride-0 SBUF->SBUF DMA; no ACT ln/exp, no K=1 matmul.
  - Out-projection: heads 0+1 contract as one K=128 matmul; head 2 is
    duplicated across both partition halves and row-packed as two
    concurrent K=64 matmuls.
"""

import contextlib
import ctypes
import os
import sys
import types

import numpy as np

B = 2
T = 2048
D = 768
NPATCH = 64
HEADS = 12
DH = 64
NH = 3          # heads per core
CH = 3 * NH * DH  # 576 qkv channels per core
LN_EPS = 1e-5
NCORES = 8

_CACHE = {}


def _install_axon_hooks_shim():
    """This image's antenv lacks axon_hooks; synthesize it so that
    run_bass_kernel_spmd(trace=True) finds the NTFF profile hook instead of
    crashing on import.  Safe no-op if profiling symbols are unavailable."""
    if "antenv.axon_hooks" in sys.modules:
        return
    mod = types.ModuleType("antenv.axon_hooks")
    _hook = [None]
    mod.set_axon_ntff_profile_hook = lambda h: _hook.__setitem__(0, h)
    mod.get_axon_ntff_profile_hook = lambda: _hook[0]
    sys.modules["antenv.axon_hooks"] = mod
    try:
        lib = ctypes.CDLL("/opt/axon/libaxon_pjrt.so")
        if not hasattr(lib, "axon_start_nrt_profile"):
            return
        lib.axon_start_nrt_profile.argtypes = [
            ctypes.POINTER(ctypes.c_int64),
            ctypes.c_size_t,
        ]
        lib.axon_start_nrt_profile.restype = ctypes.c_int64
        lib.axon_stop_nrt_profile.argtypes = [ctypes.c_char_p]
        lib.axon_stop_nrt_profile.restype = ctypes.c_int64

        @contextlib.contextmanager
        def _hook_cm(output_dir, device_ids):
            import jax

            jax.devices()
            if device_ids:
                ids = (ctypes.c_int64 * len(device_ids))(*device_ids)
                rc = lib.axon_start_nrt_profile(ids, len(device_ids))
            else:
                rc = lib.axon_start_nrt_profile(None, 0)
            if rc != 0:
                raise RuntimeError(f"axon_start_nrt_profile rc={rc}")
            try:
                yield
            finally:
                n = lib.axon_stop_nrt_profile(str(output_dir).encode())
                print(f"profile: {n} file(s) -> {output_dir}", file=sys.stderr)

        mod.set_axon_ntff_profile_hook(_hook_cm)
    except OSError:
        pass


def _install_drain_split():
    """The walrus build in this container accepts only ONE sync wait per
    CTRL(drain) instruction; Tile's tail drain carries several.  Split the
    waits across a chain of drains."""
    import bass_rust
    import concourse.tile as tile
    from concourse.vector_clock import ScopedClock

    if getattr(tile.TileContext, "_drain_split_installed", False):
        return

    def _drain_and_barrier(self, tick_clock, wait_clock):
        nc = self.nc
        drain_inst = nc.sync.drain()
        wait_clock.add_sem_waits(
            drain_inst.ins, ScopedClock({None: tick_clock.global_clock})
        )
        si = drain_inst.ins.sync_info
        if si is not None:
            waits = list(si.on_wait)
            if len(waits) > 1:
                si.on_wait = waits[:1]
                for w in waits[1:]:
                    extra = nc.sync.drain()
                    extra.ins.sync_info = bass_rust.SyncInfo(
                        on_wait=[w], on_update=[]
                    )
        nc.all_engine_barrier()
        popped = nc._tile_sem_poison_stack.pop()
        assert popped is self._sem_poison
        nc.clear_and_free_semaphores(list(self.sems.allocated().values()))
        nc.all_engine_barrier()

    tile.TileContext._drain_and_barrier = _drain_and_barrier

    # Generic pass: walrus here allows 1 sync wait per instruction; move
    # extra waits onto nofuse NOPs inserted just before, on the same engine.
    from concourse import mybir

    orig_lower = tile.TileContext._lower_ordered_insts

    def _lower_split(self, ordered):
        for insts in ordered.values():
            new = []
            for inst in insts:
                si = getattr(inst, "sync_info", None)
                eng = getattr(inst, "engine", None)
                if si is not None and eng is not None:
                    waits = list(si.on_wait)
                    if len(waits) > 1:
                        movable = [w for w in waits
                                   if getattr(w, "sync_type", "") == "semaphore"]
                        keep = [w for w in waits if w not in movable]
                        if not keep:
                            keep = [movable.pop()]
                        for k, w in enumerate(movable):
                            nop = mybir.InstNoOp(
                                name=f"{inst.name}-wsplit{k}",
                                sync_info=mybir.SyncInfo(
                                    on_wait=[w], on_update=[]
                                ),
                                bass_nofuse=True,
                                engine=eng,
                            )
                            new.append(nop)
                        inst.sync_info = mybir.SyncInfo(
                            on_wait=keep, on_update=list(si.on_update)
                        )
                new.append(inst)
            insts[:] = new
        return orig_lower(self, ordered)

    tile.TileContext._lower_ordered_insts = _lower_split
    tile.TileContext._drain_split_installed = True


# qkvT tile layout (5 tiles of [128, T]; tile 4 uses rows 0-63 only):
#   t0 = [q0; q1]   t1 = [k0; k1]   t2 = [v0; q2]   t3 = [v1; k2]
#   t4 = [v2; --]
# q/k of head h live at the SAME partition offset (matmul base-partition
# rule); h2 at offset 64 so its score matmul row-packs against h0's.
Q_LOC = [(0, 0), (0, 64), (2, 64)]
K_LOC = [(1, 0), (1, 64), (3, 64)]
V_LOC = [(2, 0), (3, 0), (4, 0)]   # vT strips (d on partitions)
# host w_qkv column order, 64-col segments
SEG_ORDER = [("q", 0), ("q", 1), ("k", 0), ("k", 1), ("v", 0),
             ("q", 2), ("v", 1), ("k", 2), ("v", 2)]

# (w-col offset, rows, qkvT tile, tile row offset)
C_CHUNKS = [(0, 128, 0, 0), (128, 128, 1, 0), (256, 128, 2, 0),
            (384, 128, 3, 0), (512, 64, 4, 0)]


def build_nc():
    import concourse.bass as bass
    import concourse.tile as tile
    from concourse import mybir
    from concourse.masks import make_identity

    _install_drain_split()

    f32 = mybir.dt.float32
    bf16 = mybir.dt.bfloat16
    AF = mybir.ActivationFunctionType
    Alu = mybir.AluOpType
    SCALE = float(DH) ** -0.5

    stop_after = os.environ.get("KV2_STOP", "")

    nc = bass.Bass()
    x_d = nc.dram_tensor("x", [T, D], f32, kind="ExternalInput")
    wqkv_d = nc.dram_tensor("wqkv", [D, CH], bf16, kind="ExternalInput")
    wout01_d = nc.dram_tensor("wout01", [128, D], bf16, kind="ExternalInput")
    wout2_d = nc.dram_tensor("wout2", [128, 512], bf16, kind="ExternalInput")
    bw_d = nc.dram_tensor("bw", [128, 5], f32, kind="ExternalInput")
    out_d = nc.dram_tensor("out", [T, D], f32, kind="ExternalOutput")
    dbg_d = None
    if stop_after:
        dbg_d = nc.dram_tensor("dbg", [6 * 128, T], bf16,
                               kind="ExternalOutput")

    with contextlib.ExitStack() as ctx:
        ctx.enter_context(
            nc.allow_low_precision(reason="bf16 PE inputs are intentional")
        )
        tc = ctx.enter_context(tile.TileContext(nc))
        consts = ctx.enter_context(tc.tile_pool(name="consts", bufs=1))
        wpool = ctx.enter_context(tc.tile_pool(name="w", bufs=1))
        qkvT_pool = ctx.enter_context(tc.tile_pool(name="qkvT", bufs=1))
        vaug_pool = ctx.enter_context(tc.tile_pool(name="vaug", bufs=1))
        ocat_pool = ctx.enter_context(tc.tile_pool(name="ocat", bufs=1))
        io_pool = ctx.enter_context(tc.tile_pool(name="io", bufs=3))
        stats = ctx.enter_context(tc.tile_pool(name="stats", bufs=4))
        fin_pool = ctx.enter_context(tc.tile_pool(name="fin", bufs=2))

        identity = consts.tile([128, 128], f32, tag="id")
        make_identity(nc, identity)
        id_bf = consts.tile([128, 128], bf16, tag="idbf")
        nc.vector.tensor_copy(id_bf, identity)
        # [I64; I64] stack: transpose rhs for 64-row strips at either offset
        idstack = consts.tile([128, 64], bf16, tag="ids")
        nc.vector.tensor_copy(idstack[0:64, :], id_bf[0:64, 0:64])
        nc.vector.tensor_copy(idstack[64:128, :], id_bf[64:128, 64:128])
        eps_t = consts.tile([128, 1], f32, tag="eps")
        nc.vector.memset(eps_t, LN_EPS)

        # weights arrive pre-folded/permuted/bf16 from the host
        w_sb = []
        for j in range(6):
            wt = wpool.tile([128, CH], bf16, tag=f"w{j}", name=f"w{j}")
            nc.sync.dma_start(wt, wqkv_d[128 * j : 128 * (j + 1), :])
            w_sb.append(wt)
        wout01 = wpool.tile([128, D], bf16, tag="wo01")
        nc.sync.dma_start(wout01, wout01_d[:, :])
        wout2 = wpool.tile([128, 512], bf16, tag="wo2")
        nc.sync.dma_start(wout2, wout2_d[:, :])
        bw_sb = consts.tile([128, 5], f32, tag="bw")
        nc.sync.dma_start(bw_sb, bw_d[:, :])

        qkvT = [qkvT_pool.tile([128, T], bf16, tag=f"qkvT{i}", name=f"qkvT{i}")
                for i in range(5)]
        nc.vector.memset(qkvT[4][64:128, :], 0.0)  # unused half stays defined
        # v natural: [keys, J, head, 2d]; cols 64-127 all ones, so the A@V
        # matmul emits the softmax denominator pre-replicated on rows 64-127
        vaug = vaug_pool.tile([128, 16, NH, 2 * DH], bf16, tag="va")
        nc.vector.memset(vaug[:, :, :, DH : 2 * DH], 1.0)
        ocat01 = ocat_pool.tile([128, T], bf16, tag="oc01")
        ocat2 = ocat_pool.tile([128, T], bf16, tag="oc2")

        # ---- Phase A: LayerNorm -> transpose (id-matmul) -> QKV -> v strips
        with (
            tc.tile_pool(name="xn", bufs=2) as xn_pool,
            tc.tile_pool(name="xnT", bufs=2) as xnT_pool,
            tc.tile_pool(name="xp_ps", bufs=2, space="PSUM") as xp_ps,
            tc.tile_pool(name="qkv_ps", bufs=2, space="PSUM") as qkv_ps,
            tc.tile_pool(name="vt_ps", bufs=2, space="PSUM") as vt_ps,
        ):
            for g in range(4):
                xts = []
                for u in range(4):
                    i = 4 * g + u
                    xt = io_pool.tile([128, D], f32, tag="xin", name="xin")
                    nc.sync.dma_start(xt, x_d[128 * i : 128 * (i + 1), :])
                    st = stats.tile([128, 3, 6], f32, tag="bnst", name="bnst")
                    for s in range(3):
                        nc.vector.bn_stats(
                            st[:, s, :], xt[:, 256 * s : 256 * (s + 1)]
                        )
                    mv = stats.tile([128, 2], f32, tag="mv", name="mv")
                    nc.vector.bn_aggr(mv, st)
                    rstd = stats.tile([128, 1], f32, tag="rstd", name="rstd")
                    nc.scalar.activation(rstd, mv[:, 1:2], AF.Sqrt, bias=eps_t)
                    nc.vector.reciprocal(rstd, rstd)
                    xn_t = xn_pool.tile([128, D], bf16, tag=f"xn{u}", name=f"xn{u}")
                    nc.vector.tensor_scalar(
                        out=xn_t,
                        in0=xt,
                        scalar1=mv[:, 0:1],
                        scalar2=rstd,
                        op0=Alu.subtract,
                        op1=Alu.mult,
                    )
                    xts.append(xn_t)
                # x^T via regular id-matmuls (pipelined, HAM-visible)
                xnT = []
                for j in range(6):
                    ps = xp_ps.tile([128, 512], f32, tag="xp", name="xp")
                    for u in range(4):
                        nc.tensor.matmul(
                            ps[:, 128 * u : 128 * (u + 1)],
                            xts[u][:, 128 * j : 128 * (j + 1)],
                            id_bf,
                            start=True,
                            stop=True,
                        )
                    xt_j = xnT_pool.tile([128, 512], bf16, tag=f"xT{j}",
                                         name=f"xT{j}")
                    nc.scalar.copy(xt_j, ps)
                    xnT.append(xt_j)
                if stop_after == "T1" and g == 0:
                    for j in range(6):
                        nc.sync.dma_start(
                            dbg_d[128 * j : 128 * (j + 1), 0:512], xnT[j]
                        )
                    return nc
                # QKV for this 512-token group
                for ci, (clo, csz, ti, ro) in enumerate(C_CHUNKS):
                    pq = qkv_ps.tile([128, 512], f32, tag="qk", name="qk")
                    for j in range(6):
                        nc.tensor.matmul(
                            pq[ro : ro + csz, :],
                            w_sb[j][:, clo : clo + csz],
                            xnT[j],
                            start=(j == 0),
                            stop=(j == 5),
                        )
                    nc.vector.tensor_scalar_add(
                        qkvT[ti][ro : ro + csz, 512 * g : 512 * (g + 1)],
                        in0=pq[ro : ro + csz, :],
                        scalar1=bw_sb[ro : ro + csz, ci : ci + 1],
                    )
                if stop_after == "Q1" and g == 0:
                    for i in range(5):
                        nc.sync.dma_start(
                            dbg_d[128 * i : 128 * (i + 1), 0:512],
                            qkvT[i][:, 0:512],
                        )
                    return nc
                # v natural strips for this group's 4 key blocks
                vt_mode = os.environ.get("KV2_VT", "3")
                for u in range(4):
                    J = 4 * g + u
                    # [128, 512] so each psum buffer is bank-aligned
                    pv = vt_ps.tile([128, 512], f32, tag="vt", name="vt")
                    sel = {"1": [0], "2": [0, 1, 2], "2a": [0, 1],
                           "2b": [0, 2]}.get(vt_mode, [0, 1, 2])
                    for h in sel:
                        ti, ro = V_LOC[h]
                        nc.tensor.matmul(
                            pv[:, DH * h : DH * (h + 1)],
                            qkvT[ti][ro : ro + 64, 128 * J : 128 * (J + 1)],
                            idstack[ro : ro + 64, :],
                            start=True,
                            stop=True,
                        )
                    if vt_mode not in ("1", "2", "2a", "2b"):
                        nc.vector.tensor_copy(
                            vaug[:, J, :, 0:DH],
                            pv[:, 0 : NH * DH].rearrange(
                                "p (h d) -> p h d", h=NH),
                        )
                if stop_after == "V1" and g == 0:
                    nc.sync.dma_start(dbg_d[0:128, :], qkvT[0][:, :])
                    return nc

        if stop_after == "A":
            for i in range(5):
                nc.sync.dma_start(dbg_d[128 * i : 128 * (i + 1), :], qkvT[i])
            return nc

        # ---- Phase B: scores -> exp -> A@V -> normalize (per 512-q chunk)
        with (
            tc.tile_pool(name="st_ps", bufs=1, space="PSUM") as st_ps,
            tc.tile_pool(name="ot_ps", bufs=1, space="PSUM") as ot_ps,
            tc.tile_pool(name="pt", bufs=1) as pt_pool,
            tc.tile_pool(name="otmp", bufs=2) as otmp_pool,
        ):
            for c in range(4):
                otp = [ot_ps.tile([128, 512], f32, tag=f"ot{h}",
                                  name=f"ot{h}") for h in range(NH)]
                nJ = 4 * c + 4
                pending = []

                def emit_av(Jp, s0p, np_, srcs, otp=otp, nJ=nJ):
                    for h in range(NH):
                        nc.tensor.matmul(
                            otp[h][:, s0p:512],
                            vaug[:, Jp, h, :],
                            srcs[h],
                            start=(Jp == 0),
                            stop=(Jp == nJ - 1),
                        )

                psA = ptA = None
                for J in range(nJ):
                    s0 = max(0, 128 * J - 512 * c)
                    n = 512 - s0
                    q0 = 512 * c + s0
                    even = (J % 2 == 0)
                    if even:
                        psA = st_ps.tile([128, 3 * 512], f32, tag="sA",
                                         name="sA")
                        ptA = pt_pool.tile([128, 3 * 512], bf16, tag="ptA",
                                           name="ptA")
                        ps01 = psA
                        pt01 = ptA
                    else:
                        ps01 = st_ps.tile([128, 2 * 512], f32, tag="sB",
                                          name="sB")
                        pt01 = pt_pool.tile([128, 2 * 512], bf16, tag="ptB",
                                            name="ptB")
                    # scores: h1 (hi), h0 (lo, concurrent), h2 (hi)
                    for h in (1, 0, 2):
                        qt, qo = Q_LOC[h]
                        kt, ko = K_LOC[h]
                        dst = psA if h == 2 else ps01
                        col = 1024 if h == 2 else 512 * h
                        nc.tensor.matmul(
                            dst[:, col : col + n],
                            qkvT[kt][ko : ko + 64, 128 * J : 128 * (J + 1)],
                            qkvT[qt][qo : qo + 64, q0 : q0 + n],
                            start=True,
                            stop=True,
                        )
                    if pending:
                        emit_av(*pending.pop(0))
                    # exp: fused ACTIVATEs on interior J (all cols written),
                    # per-head exact ranges on diagonal J (avoid stale psum)
                    if n == 512:
                        if even:
                            nc.scalar.activation(
                                ptA[:, 0:1536], psA[:, 0:1536],
                                AF.Exp, scale=SCALE,
                            )
                        else:
                            nc.scalar.activation(
                                pt01[:, 0:1024], ps01[:, 0:1024],
                                AF.Exp, scale=SCALE,
                            )
                            nc.scalar.activation(
                                ptA[:, 1024:1536], psA[:, 1024:1536],
                                AF.Exp, scale=SCALE,
                            )
                    else:
                        nc.scalar.activation(
                            pt01[:, 0:n], ps01[:, 0:n], AF.Exp, scale=SCALE,
                        )
                        nc.scalar.activation(
                            pt01[:, 512 : 512 + n], ps01[:, 512 : 512 + n],
                            AF.Exp, scale=SCALE,
                        )
                        nc.scalar.activation(
                            ptA[:, 1024 : 1024 + n], psA[:, 1024 : 1024 + n],
                            AF.Exp, scale=SCALE,
                        )
                    if J >= 4 * c:
                        # block-causal corner: first 64 queries vs odd window
                        nc.vector.memset(pt01[64:128, 0:64], 0.0)
                        nc.vector.memset(pt01[64:128, 512 : 512 + 64], 0.0)
                        nc.vector.memset(ptA[64:128, 1024 : 1024 + 64], 0.0)
                    srcs = [pt01[:, 0:n], pt01[:, 512 : 512 + n],
                            ptA[:, 1024 : 1024 + n]]
                    pending.append((J, s0, n, srcs))
                while pending:
                    emit_av(*pending.pop(0))

                # finalize chunk: den is pre-replicated on psum rows 64-127
                # by the ones half of vaug; reciprocal it there (aligned),
                # DMA-shift to partitions 0-63, divide during psum eviction
                rec_hi = fin_pool.tile([128, 3 * 512], f32, tag="rechi",
                                       name="rechi")
                recs = fin_pool.tile([64, 3 * 512], f32, tag="recs",
                                     name="recs")
                for h in range(NH):
                    sl = slice(512 * h, 512 * h + 512)
                    nc.vector.reciprocal(rec_hi[64:128, sl],
                                         otp[h][DH : 2 * DH, :])
                nc.sync.dma_start(recs[:, :], rec_hi[64:128, :])
                cs = slice(512 * c, 512 * (c + 1))
                nc.vector.tensor_mul(
                    ocat01[0:64, cs], otp[0][0:DH, :], recs[:, 0:512]
                )
                ot1 = otmp_pool.tile([64, 512], bf16, tag="ot1s", name="ot1s")
                nc.vector.tensor_mul(
                    ot1, otp[1][0:DH, :], recs[:, 512:1024]
                )
                nc.sync.dma_start(ocat01[64:128, cs], ot1)
                nc.vector.tensor_mul(
                    ocat2[0:64, cs], otp[2][0:DH, :], recs[:, 1024:1536]
                )
                nc.sync.dma_start(ocat2[64:128, cs], ocat2[0:64, cs])

        if stop_after == "B":
            nc.sync.dma_start(dbg_d[0:128, :], ocat01)
            nc.sync.dma_start(dbg_d[128:256, :], ocat2)
            return nc

        # ---- Phase C: out-projection
        with (
            tc.tile_pool(name="op_ps", bufs=2, space="PSUM") as op_ps,
        ):
            for t in range(16):
                tsl = slice(128 * t, 128 * (t + 1))
                # [128, 1024] so each psum buffer is bank-aligned
                opp = op_ps.tile([128, 1024], f32, tag="op", name="op")
                nc.tensor.matmul(opp[:, 0:512], ocat01[:, tsl],
                                 wout01[:, 0:512], start=True, stop=False,
                                 skip_group_check=True)
                nc.tensor.matmul(opp[:, 512:768], ocat01[:, tsl],
                                 wout01[:, 512:768], start=True, stop=False,
                                 skip_group_check=True)
                nc.tensor.matmul(opp[:, 0:512], ocat2[0:64, tsl],
                                 wout2[0:64, :], start=False, stop=True,
                                 skip_group_check=True)
                nc.tensor.matmul(opp[:, 512:768], ocat2[64:128, tsl],
                                 wout2[64:128, 0:256], start=False, stop=True,
                                 skip_group_check=True)
                ot_sb = io_pool.tile([128, D], f32, tag="osb", name="osb")
                nc.scalar.copy(ot_sb, opp[:, 0:D])
                nc.sync.dma_start(out_d[tsl, :], ot_sb)

    return nc


def shard_inputs(x, gamma, beta, w_qkv, w_out, b_out):
    """Full inputs -> list of 8 per-core input dicts (host-side folding)."""
    import ml_dtypes

    bf16 = ml_dtypes.bfloat16
    x = np.ascontiguousarray(np.asarray(x, dtype=np.float32))
    gamma = np.asarray(gamma, dtype=np.float32)
    beta = np.asarray(beta, dtype=np.float32)
    w_qkv = np.asarray(w_qkv, dtype=np.float32)
    w_out = np.asarray(w_out, dtype=np.float32)
    wg = w_qkv * gamma[:, None]          # gamma folded into the projection
    bw_full = beta @ w_qkv               # [3*inner] bias from beta
    in_maps = []
    for g in range(NCORES):
        b = g // 4
        hg = g % 4
        heads = [3 * hg + h for h in range(NH)]
        segs = []
        segb = []
        for kind, h in SEG_ORDER:
            hh = heads[h]
            base = {"q": 0, "k": D, "v": 2 * D}[kind]
            segs.append(wg[:, base + 64 * hh : base + 64 * (hh + 1)])
            segb.append(bw_full[base + 64 * hh : base + 64 * (hh + 1)])
        wqkv_g = np.ascontiguousarray(
            np.concatenate(segs, axis=1).astype(bf16)
        )
        bw_g = np.zeros((128, 5), np.float32)
        for ci, (clo, csz, ti, ro) in enumerate(C_CHUNKS):
            bias = np.concatenate(segb[clo // 64 : (clo + csz) // 64])
            bw_g[ro : ro + csz, ci] = bias
        wo = [w_out[64 * hh : 64 * (hh + 1), :] for hh in heads]
        wout01_g = np.ascontiguousarray(
            np.concatenate([wo[0], wo[1]], axis=0).astype(bf16)
        )
        wout2_g = np.zeros((128, 512), np.float32)
        wout2_g[0:64, :] = wo[2][:, 0:512]
        wout2_g[64:128, 0:256] = wo[2][:, 512:768]
        in_maps.append(
            {
                "x": x[b],
                "wqkv": wqkv_g,
                "wout01": wout01_g,
                "wout2": np.ascontiguousarray(wout2_g.astype(bf16)),
                "bw": bw_g,
            }
        )
    return in_maps


def kernel(x, gamma, beta, w_qkv, w_out, b_out):
    _install_axon_hooks_shim()
    from concourse import bass_utils

    if "nc" not in _CACHE:
        _CACHE["nc"] = build_nc()
    nc = _CACHE["nc"]

    in_maps = shard_inputs(x, gamma, beta, w_qkv, w_out, b_out)
    trace = bool(int(os.environ.get("KERNEL_TRACE", "0")))
    kwargs = {}
    if trace:
        kwargs["trace"] = True
        tmpdir = os.environ.get("KERNEL_TRACE_DIR")
        if tmpdir:
            kwargs["tmpdir"] = tmpdir
        # artifact upload needs external storage; keep the trace local
        bass_utils.upload_artifacts = lambda d: d
    res = bass_utils.run_bass_kernel_spmd(
        nc, in_maps, list(range(NCORES)), **kwargs
    )
    _CACHE["last_exec_time_ns"] = res.exec_time_ns

    b_out = np.asarray(b_out, dtype=np.float32)
    out = np.empty((B, T, D), dtype=np.float32)
    for b in range(B):
        acc = res.results[4 * b]["out"].astype(np.float32)
        for hg in range(1, 4):
            acc = acc + res.results[4 * b + hg]["out"]
        out[b] = acc + b_out[None, :]
    return out
